# revision 10
# baseline (speedup 1.0000x reference)
"""Bayesian STDP spiking WTA network — Trainium2 Bass kernel.

Self-contained: hardcodes shapes T=1000, B=32, Nin=1024, Nout=256.
Runs the full sequential recurrence SPMD-replicated on 8 NeuronCores
(identical work per core, deterministic); core 0's outputs are returned.
Returns (z_acc [T,B,Nout] f32, v [B,Nout] f32).

Design notes:
  - forward matmul in fp32, vT layout (W^T chunks stationary, xT moving),
    then PE-transpose z_in.T -> [B, Nout]; fp32 keeps the trajectory within
    the reference's argmax-flip noise floor.
  - spikes are PE-transposed once into a bf16 DRAM staging tensor
    (T-layout), read back one 8-step block at a time.
  - gumbel trick: argmax(logsoftmax(v)+g) == argmax(v - ln(-ln u)).
  - STDP update: dense exp(-W) + dense DVE update (phase 1).
"""
import numpy as np

T, B, NIN, NOUT = 1000, 32, 1024, 256
P = 128
NCH = NIN // P            # 8 Nin chunks
KH = NOUT // P            # 2 Nout halves
PSP_DECAY = float(np.exp(-1.0 / 10.0))
BETA = 0.9
LR = 0.01
C_W = 1.0
U = 8                     # step unroll inside For_i (block = U*B staged rows)
RB = U * B                # 256
NBLK = T * B // RB        # 125
N_CORES = 8

_CACHE = {}


def _build():
    import concourse.tile as tile
    from concourse import bacc, mybir
    import concourse.bass as bass
    from concourse.masks import make_identity
    import contextlib

    dt = mybir.dt
    A = mybir.AluOpType
    AF = mybir.ActivationFunctionType

    nc = bacc.Bacc("TRN2", target_bir_lowering=False, debug=False,
                   num_devices=N_CORES)

    spikes_d = nc.dram_tensor("input_spikes", [T, B, NIN], dt.float32, kind="ExternalInput").ap()
    noise_d = nc.dram_tensor("noise_u", [T, B, NOUT], dt.float32, kind="ExternalInput").ap()
    weight_d = nc.dram_tensor("weight", [NOUT, NIN], dt.float32, kind="ExternalInput").ap()
    bias_d = nc.dram_tensor("bias", [NOUT], dt.float32, kind="ExternalInput").ap()
    zacc_d = nc.dram_tensor("z_acc", [T, B, NOUT], dt.float32, kind="ExternalOutput").ap()
    vout_d = nc.dram_tensor("v_out", [B, NOUT], dt.float32, kind="ExternalOutput").ap()
    staged_d = nc.dram_tensor("staged_sT", [P, NBLK, NCH, RB], dt.bfloat16, kind="Internal").ap()

    spikes_flat = spikes_d.rearrange("t b i -> (t b) i")

    with tile.TileContext(nc) as tc:
        with contextlib.ExitStack() as ctx:
            st = ctx.enter_context(tc.tile_pool(name="st", bufs=1))
            dp = ctx.enter_context(tc.tile_pool(name="dp", bufs=3))
            tp = ctx.enter_context(tc.tile_pool(name="tp", bufs=2))
            pz = ctx.enter_context(tc.tile_pool(name="pz", bufs=1, space="PSUM"))

            # ---------------- persistent state ----------------
            WT_s = st.tile([P, NCH * NOUT], dt.float32)    # W^T master [p, c*256+k]
            E_s = st.tile([P, NCH * NOUT], dt.float32)     # exp(-W)
            X1_s = st.tile([P, NCH * NOUT], dt.float32)
            zkrow = st.tile([P, NOUT], dt.float32)
            xT_s = st.tile([P, NCH, B], dt.float32)        # PSP T-layout [p, c, b]
            x16_s = st.tile([B, NIN], dt.float16)          # PSP row-layout (low precision path)
            zinT_sb = st.tile([P, KH, B], dt.float32)
            v_s = st.tile([B, NOUT], dt.float32)
            s_s = st.tile([B, NOUT], dt.float32)
            a_s = st.tile([B, NOUT], dt.float32)
            c_s = st.tile([B, NOUT], dt.float32)
            bT_s = st.tile([P, KH], dt.float32)            # bias, T-layout [k%128, k//128]
            ebT_s = st.tile([P, KH], dt.float32)
            btT_s = st.tile([P, KH], dt.float32)
            zT_sb = st.tile([P, KH, B], dt.float16)
            zkT_s = st.tile([P, KH], dt.float32)
            mx_s = st.tile([B, 8], dt.float32)
            mi_s = st.tile([B, 8], dt.uint32)
            kf_s = st.tile([B, 1], dt.float32)
            z16_s = st.tile([B, NOUT], dt.float16)
            z32_s = st.tile([B, NOUT], dt.float32)
            iota_i = st.tile([B, NOUT], dt.int32)
            iota_f = st.tile([B, NOUT], dt.float32)
            ident32 = st.tile([P, P], dt.float32)
            identb = st.tile([P, P], dt.bfloat16)
            identB16 = st.tile([B, B], dt.float16)
            onesBh = st.tile([B, P], dt.float16)

            # constants
            nc.gpsimd.iota(iota_i[:], pattern=[[1, NOUT]], base=0, channel_multiplier=0)
            nc.vector.tensor_copy(iota_f[:], iota_i[:])
            make_identity(nc, ident32[:])
            nc.vector.tensor_copy(identb[:], ident32[:])
            nc.vector.tensor_copy(identB16[:], ident32[0:B, 0:B])
            nc.gpsimd.memset(onesBh[:], 1.0)

            # zero init
            nc.vector.memset(xT_s[:], 0.0)
            nc.vector.memset(x16_s[:], 0.0)
            nc.vector.memset(v_s[:], 0.0)

            # ---------------- weight / bias prep ----------------
            # WT_s[p, c*NOUT + h*128 + k'] = weight[h*128 + k', c*128 + p]
            for h in range(KH):
                wrow_g = tp.tile([P, NIN], dt.float32, tag="wrow")
                nc.sync.dma_start(wrow_g[:], weight_d[h * P:(h + 1) * P, :])
                wrow = tp.tile([P, NIN], dt.float32, tag="wrow2")
                nc.vector.tensor_copy(wrow[:], wrow_g[:])
                for c in range(NCH):
                    wT_ps = pz.tile([P, P], dt.float32, tag="zk", name="wT_ps")
                    nc.tensor.transpose(wT_ps[:], wrow[:, c * P:(c + 1) * P], ident32[:])
                    nc.vector.tensor_copy(WT_s[:, c * NOUT + h * P: c * NOUT + (h + 1) * P], wT_ps[:])

            # bias -> T-layout [p, h]
            bg = tp.tile([1, NOUT], dt.float32, tag="bg")
            nc.sync.dma_start(bg[:], bias_d[:].rearrange("(o k) -> o k", o=1))
            bg2 = tp.tile([1, NOUT], dt.float32, tag="bg2")
            nc.vector.tensor_copy(bg2[:], bg[:])
            for h in range(KH):
                bT_ps = pz.tile([P, 1], dt.float32, tag="zinT", name="bT_ps")
                nc.tensor.transpose(bT_ps[:], bg2[:, h * P:(h + 1) * P], ident32[0:1, 0:1])
                nc.vector.tensor_copy(bT_s[:, h:h + 1], bT_ps[:])

            # ---------------- spike transpose preprocessing ----------------
            with tc.For_i(0, NBLK, 1, name="prep") as g:
                blk_sb = tp.tile([P, NCH, RB], dt.bfloat16, tag="blk")
                srows = []
                for j in range(2):
                    srow_g = dp.tile([P, NIN], dt.float32, tag=f"prow{j}", name=f"prow{j}")
                    nc.gpsimd.dma_start(
                        srow_g[:], spikes_flat[bass.ds(g * RB + j * P, P), :])
                    srow_b = tp.tile([P, NIN], dt.bfloat16, tag=f"prowb{j}", name=f"prowb{j}")
                    nc.gpsimd.tensor_copy(srow_b[:], srow_g[:])
                    srows.append(srow_b)
                for cc in range(NCH // 2):
                    ps_t = pz.tile([P, 2, RB], dt.bfloat16, tag=f"zx{cc}", name=f"psT{cc}")
                    for c2 in range(2):
                        c = cc * 2 + c2
                        for j in range(2):
                            nc.tensor.transpose(ps_t[:, c2, j * P:(j + 1) * P],
                                                srows[j][:, c * P:(c + 1) * P], identb[:])
                    nc.vector.tensor_copy(blk_sb[:, cc * 2:(cc + 1) * 2, :], ps_t[:])
                nc.gpsimd.dma_start(staged_d[:, bass.ds(g, 1), :, :],
                                    blk_sb[:].rearrange("p (o c) r -> p o c r", o=1))
                for eng in (nc.tensor, nc.vector, nc.scalar, nc.gpsimd, nc.sync):
                    eng.nop()

            # ---------------- main loop ----------------
            assert T % U == 0
            with tc.For_i(0, T // U, 1, name="steps") as it:
                sT_blk = dp.tile([P, NCH, RB], dt.bfloat16, tag="sTb")
                nc.gpsimd.dma_start(sT_blk[:].rearrange("p (o c) r -> p o c r", o=1),
                                    staged_d[:, bass.ds(it, 1), :, :])
                for u in range(U):
                    t_sv = it * U + u

                    u_t = dp.tile([B, NOUT], dt.float32, tag="ut")
                    nc.gpsimd.dma_start(
                        u_t[:], noise_d[bass.ds(t_sv, 1), :, :].rearrange("o b k -> (o b) k"))
                    srow_t = dp.tile([B, NIN], dt.float32, tag="srow")
                    nc.gpsimd.dma_start(
                        srow_t[:], spikes_d[bass.ds(t_sv, 1), :, :].rearrange("o b i -> (o b) i"))

                    # --- PSP updates ---
                    nc.vector.scalar_tensor_tensor(
                        xT_s[:], xT_s[:], PSP_DECAY, sT_blk[:, :, u * B:(u + 1) * B],
                        op0=A.mult, op1=A.add)
                    nc.vector.scalar_tensor_tensor(
                        x16_s[:], x16_s[:], PSP_DECAY, srow_t[:], op0=A.mult, op1=A.add)

                    # --- gumbel c = ln(-ln u) ---
                    nc.scalar.activation(a_s[:], u_t[:], AF.Ln)
                    nc.scalar.activation(c_s[:], a_s[:], AF.Ln, scale=-1.0)

                    # --- forward: z_in.T = W @ x.T (fp32), + b via T-layout add ---
                    p_zinT = pz.tile([P, KH, B], dt.float32, tag="zinT")
                    for h in range(KH):
                        for c in range(NCH):
                            wsl = slice(c * NOUT + h * P, c * NOUT + (h + 1) * P)
                            nc.tensor.matmul(p_zinT[:, h, :], WT_s[:, wsl], xT_s[:, c, :],
                                             start=(c == 0), stop=(c == NCH - 1))
                    nc.vector.tensor_tensor(
                        zinT_sb[:], p_zinT[:],
                        bT_s[:].rearrange("p (h o) -> p h o", o=1).broadcast_to([P, KH, B]),
                        op=A.add)
                    # transpose z_in.T -> z_in [B, NOUT]
                    p_zin = pz.tile([B, NOUT], dt.float32, tag="zin")
                    for h in range(KH):
                        nc.tensor.transpose(p_zin[:, h * P:(h + 1) * P], zinT_sb[:, h, :], ident32[:])

                    # --- membrane + WTA sample ---
                    nc.vector.scalar_tensor_tensor(v_s[:], v_s[:], BETA, p_zin[:],
                                                   op0=A.mult, op1=A.add)
                    nc.vector.tensor_tensor(s_s[:], v_s[:], c_s[:], op=A.subtract)
                    nc.vector.max(mx_s[:], s_s[:])
                    nc.vector.max_index(mi_s[:], mx_s[:], s_s[:])
                    nc.vector.tensor_copy(kf_s[:], mi_s[:, 0:1])
                    nc.vector.tensor_scalar(z16_s[:], iota_f[:], kf_s[:, 0:1], None, op0=A.is_equal)
                    nc.vector.tensor_scalar(z32_s[:], iota_f[:], kf_s[:, 0:1], None, op0=A.is_equal)

                    nc.gpsimd.dma_start(
                        zacc_d[bass.ds(t_sv, 1), :, :].rearrange("o b k -> (o b) k"), z32_s[:])

                    # z.T (f16) for zkT (and later sparse paths)
                    p_zT = pz.tile([P, KH, B], dt.float16, tag="zT")
                    for h in range(KH):
                        nc.tensor.transpose(p_zT[:, h, :], z16_s[:, h * P:(h + 1) * P], identB16[:])
                    nc.vector.tensor_copy(zT_sb[:], p_zT[:])
                    nc.vector.tensor_reduce(zkT_s[:], zT_sb[:], axis=mybir.AxisListType.X, op=A.add)

                    # --- STDP update (dense phase-1) ---
                    p_zxs = []
                    for cc in range(NCH // 2):
                        p_zx = pz.tile([P, 2, NOUT], dt.float32, tag=f"zx{cc}", name=f"pzx{cc}")
                        for c2 in range(2):
                            c = cc * 2 + c2
                            nc.tensor.matmul(p_zx[:, c2, :], x16_s[:, c * P:(c + 1) * P], z16_s[:],
                                             start=True, stop=True)
                        p_zxs.append(p_zx)

                    nc.scalar.activation(E_s[:], WT_s[:], AF.Exp, scale=-1.0)
                    for cc in range(NCH // 2):
                        nc.vector.scalar_tensor_tensor(
                            X1_s[:, cc * 2 * NOUT:(cc + 1) * 2 * NOUT],
                            E_s[:, cc * 2 * NOUT:(cc + 1) * 2 * NOUT],
                            LR * C_W / B, p_zxs[cc][:].rearrange("p c k -> p (c k)"),
                            op0=A.mult, op1=A.mult)
                    # zkrow: replicate zk along partitions via ones-matmul? reuse z-transposes:
                    # row-broadcast of zk comes from a matmul of ones with z16
                    p_zk = pz.tile([P, NOUT], dt.float32, tag="zk")
                    nc.tensor.matmul(p_zk[:], onesBh[:], z16_s[:],
                                     start=True, stop=True)
                    nc.vector.tensor_copy(zkrow[:], p_zk[:])
                    nc.vector.scalar_tensor_tensor(
                        WT_s[:].rearrange("p (c k) -> p c k", c=NCH),
                        zkrow[:].rearrange("p (o k) -> p o k", o=1).broadcast_to([P, NCH, NOUT]),
                        -LR / B,
                        WT_s[:].rearrange("p (c k) -> p c k", c=NCH),
                        op0=A.mult, op1=A.add)
                    nc.vector.tensor_tensor(WT_s[:], WT_s[:], X1_s[:], op=A.add)

                    # --- bias update (T-layout): b += LR*(C*exp(-b)*zk - 1) ---
                    nc.scalar.activation(ebT_s[:], bT_s[:], AF.Exp, scale=-1.0)
                    nc.vector.scalar_tensor_tensor(btT_s[:], ebT_s[:], LR * C_W / B, zkT_s[:],
                                                   op0=A.mult, op1=A.mult)
                    nc.vector.scalar_tensor_tensor(bT_s[:], btT_s[:], -LR, bT_s[:],
                                                   op0=A.add, op1=A.add)

            vtmp = st.tile([B, NOUT], dt.float32)
            nc.vector.tensor_copy(vtmp[:], v_s[:])
            nc.sync.dma_start(vout_d[:], vtmp[:])

    nc.compile()
    return nc


def _get_nc():
    if "nc" not in _CACHE:
        _CACHE["nc"] = _build()
    return _CACHE["nc"]


def kernel(input_spikes, noise_u, weight, bias):
    from concourse.bass_utils import run_bass_kernel_spmd

    nc = _get_nc()
    in_map = {
        "input_spikes": np.ascontiguousarray(input_spikes, dtype=np.float32),
        "noise_u": np.ascontiguousarray(noise_u, dtype=np.float32),
        "weight": np.ascontiguousarray(weight, dtype=np.float32),
        "bias": np.ascontiguousarray(bias, dtype=np.float32),
    }
    res = run_bass_kernel_spmd(nc, [in_map] * N_CORES, core_ids=list(range(N_CORES)))
    r0 = res.results[0]
    return r0["z_acc"], r0["v_out"]
